# revision 1
# baseline (speedup 1.0000x reference)
"""Trainium2 Bass kernel for nn_DisentangledGraphConvEncoder.

Strategy: channel sharding. After the input projection, the C=8 channels of
this network never interact (per-channel conv, per-channel bmm, LayerNorm over
H, relu), so core c computes channel c end-to-end on the full graph with zero
cross-core communication.

Per core (= one channel c):
  h0 = x @ proj[:, c, :]                       (PE, xT streamed from DRAM)
  for layer in (W1, W2):
      per 64-node dst group, per 128-edge chunk:
          g    = h[src[chunk]]                 (dma_gather, 256B rows)
          aggT += g.T @ S_chunk                (PE; S = omega-folded one-hot)
      out  = aggT.T @ W_c                      (PE)
      out  = LN(out) (+relu after layer 0)     (DVE bn_stats/bn_aggr + ACT)
      write to next table / output

Edges are destination-sorted on the host; each (group, src-half) run is padded
to a multiple of 128 so every chunk maps to one 64-node group and one gather
table (the int16 gather index forces a lo/hi table split at row 25600).
"""

from dataclasses import dataclass, field

import numpy as np

import concourse.bass as bass
import concourse.bacc as bacc
import concourse.tile as tile
from concourse import mybir
from concourse import bass_utils


@dataclass
class Cfg:
    N: int = 50000
    E: int = 800000
    D: int = 256
    C: int = 8
    H: int = 64
    GRP: int = 64          # one-hot width / node group size
    LO_ROWS: int = 25600   # lo gather-table rows (multiple of GRP, < 32768)
    BH: int = 64           # chunks per dma_gather call
    LN_EPS: float = 1e-5
    n_cores: int = 8
    single_packet: bool = False
    dma_scratch: int = 16384
    CALLB: int = 8         # chunks per dma_gather call (ring: scratch/16 descs)

    @property
    def NPAD(self):
        return ((self.N + self.GRP - 1) // self.GRP) * self.GRP

    @property
    def NGRP(self):
        return self.NPAD // self.GRP


FULL = Cfg()
F32 = mybir.dt.float32


# ----------------------------------------------------------------------------
# Host-side preprocessing
# ----------------------------------------------------------------------------

def _build_stream(cfg, sel, src_s, dst_s, lo):
    """Build one (half) padded edge stream.

    sel: edge positions (into dst-sorted arrays) of this half, dst-sorted.
    Returns dict with idx16 [16, L/16], chunk counts per group, and the
    (pos, col, sel) needed to fill per-core S values.
    """
    GRP = cfg.GRP
    g = dst_s[sel] // GRP
    counts = np.bincount(g, minlength=cfg.NGRP)
    return {"sel": sel, "g": g, "counts": counts, "lo": lo}


def _finish_streams(cfg, st_lo, st_hi, src_s, dst_s):
    # pad each (group, half) run to a chunk multiple; guarantee >=1 chunk per
    # group overall so every psum tile gets written (zero S rows -> zeros).
    both0 = (st_lo["counts"] == 0) & (st_hi["counts"] == 0)
    for st in (st_lo, st_hi):
        padded = ((st["counts"] + 127) // 128) * 128
        if st["lo"]:
            padded = np.maximum(padded, both0.astype(np.int64) * 128)
        st["padded"] = padded
        st["chunks"] = (padded // 128).astype(np.int64)
        starts = np.concatenate([[0], np.cumsum(padded)[:-1]])
        L = int(padded.sum())
        sel, g = st["sel"], st["g"]
        cum = np.concatenate([[0], np.cumsum(st["counts"])[:-1]])
        rank = np.arange(len(sel)) - np.repeat(cum, st["counts"])
        pos = starts[g] + rank
        idx_vals = np.zeros(L, np.int64)
        src = src_s[sel]
        idx_vals[pos] = src if st["lo"] else src - cfg.LO_ROWS
        assert idx_vals.min() >= 0 and idx_vals.max() < 32768
        st["idx16"] = np.ascontiguousarray(
            idx_vals.astype(np.int16).reshape(-1, 16).T)
        st["pos"] = pos
        st["col"] = dst_s[sel] - g * cfg.GRP
        st["L"] = L
    return st_lo, st_hi


def _stream_S(cfg, st, omega_c_sorted):
    """Per-core S array [128, n_chunks, GRP] float32 (partition-major)."""
    L = st["L"]
    S = np.zeros((L, cfg.GRP), np.float32)
    S[st["pos"], st["col"]] = omega_c_sorted[st["sel"]]
    return np.ascontiguousarray(
        S.reshape(L // 128, 128, cfg.GRP).transpose(1, 0, 2))


def preprocess(cfg, x, edge_index, omega, proj, W1, W2, ln_gamma, ln_beta):
    src = np.asarray(edge_index[0], dtype=np.int64)
    dst = np.asarray(edge_index[1], dtype=np.int64)
    omega = np.asarray(omega, dtype=np.float32)
    x = np.asarray(x, dtype=np.float32)

    order = np.argsort(dst, kind="stable")
    src_s, dst_s = src[order], dst[order]
    omega_s = omega[order]

    lo_mask = src_s < cfg.LO_ROWS
    st_lo = _build_stream(cfg, np.nonzero(lo_mask)[0], src_s, dst_s, True)
    st_hi = _build_stream(cfg, np.nonzero(~lo_mask)[0], src_s, dst_s, False)
    st_lo, st_hi = _finish_streams(cfg, st_lo, st_hi, src_s, dst_s)

    xT = np.zeros((cfg.D, cfg.NPAD), np.float32)
    xT[:, :cfg.N] = x.T

    skip_affine = bool(np.all(np.asarray(ln_gamma) == 1.0)
                       and np.all(np.asarray(ln_beta) == 0.0))

    shared = {"xT": xT, "idx_lo": st_lo["idx16"], "idx_hi": st_hi["idx16"]}
    per_core = []
    for c in range(cfg.C):
        m = dict(shared)
        m["proj"] = np.ascontiguousarray(np.asarray(proj)[:, c, :], dtype=np.float32)
        m["W1"] = np.ascontiguousarray(np.asarray(W1)[c], dtype=np.float32)
        m["W2"] = np.ascontiguousarray(np.asarray(W2)[c], dtype=np.float32)
        m["S_lo"] = _stream_S(cfg, st_lo, omega_s[:, c])
        m["S_hi"] = _stream_S(cfg, st_hi, omega_s[:, c])
        if not skip_affine:
            m["gamma"] = np.asarray(ln_gamma, dtype=np.float32).reshape(1, cfg.H)
            m["beta"] = np.asarray(ln_beta, dtype=np.float32).reshape(1, cfg.H)
        per_core.append(m)

    lo_chunks = [int(v) for v in st_lo["chunks"]]
    hi_chunks = [int(v) for v in st_hi["chunks"]]
    return per_core, lo_chunks, hi_chunks, skip_affine


# ----------------------------------------------------------------------------
# Bass program
# ----------------------------------------------------------------------------

class GatherStream:
    """Streams gather tiles + S tiles for one (layer, half)."""

    def __init__(self, nc, cfg, name, idx_dram, S_dram, n_chunks, table_view,
                 gpool, spool, ipool, join_inst=None):
        self.nc, self.cfg, self.name = nc, cfg, name
        self.idx_dram, self.S_dram = idx_dram, S_dram
        self.n_chunks = n_chunks
        self.table_view = table_view
        self.gpool, self.spool, self.ipool = gpool, spool, ipool
        self.join_inst = join_inst
        self.cur_bt = -1
        self.gtile = None
        self.stile = None

    def _issue(self, bt):
        nc, cfg = self.nc, self.cfg
        b0 = bt * cfg.BH
        B = min(cfg.BH, self.n_chunks - b0)
        ni = B * 128
        itile = self.ipool.tile([128, cfg.BH * 8], mybir.dt.int16,
                                tag="i")
        idx_src = self.idx_dram[:, b0 * 8: b0 * 8 + B * 8]
        bcast = bass.AP(tensor=idx_src.tensor, offset=idx_src.offset,
                        ap=[[0, 8]] + idx_src.ap)
        nc.sync.dma_start(out=itile[:, :B * 8], in_=bcast)

        self.gtile = self.gpool.tile([128, cfg.BH, cfg.H], F32,
                                     tag="g")
        for cb in range(0, B, cfg.CALLB):
            nb = min(cfg.CALLB, B - cb)
            nc.gpsimd.dma_gather(
                out_ap=self.gtile[:, cb:cb + nb, :], in_ap=self.table_view,
                idxs_ap=itile[:, cb * 8:(cb + nb) * 8],
                num_idxs=nb * 128, num_idxs_reg=nb * 128,
                elem_size=cfg.H, single_packet=cfg.single_packet)

        self.stile = self.spool.tile([128, cfg.BH, cfg.GRP], F32,
                                     tag="s")
        nc.sync.dma_start(out=self.stile[:, :B, :],
                          in_=self.S_dram[:, b0:b0 + B, :])
        self.cur_bt = bt

    def chunk(self, ci):
        """Returns (g_ap, s_ap) for stream chunk index ci."""
        bt, off = divmod(ci, self.cfg.BH)
        if bt != self.cur_bt:
            assert bt == self.cur_bt + 1
            self._issue(bt)
        return self.gtile[:, off, :], self.stile[:, off, :]


def build_program(cfg, lo_chunks, hi_chunks, skip_affine, num_devices=8,
                  extra_layers=0, flavor="full"):
    nc = bacc.Bacc("TRN2", target_bir_lowering=False, debug=False,
                   num_devices=num_devices,
                   dynamic_dma_scratch_size=cfg.dma_scratch)
    NPAD, H, GRP = cfg.NPAD, cfg.H, cfg.GRP
    NL, NH = sum(lo_chunks), sum(hi_chunks)

    xT = nc.dram_tensor("xT", [cfg.D, NPAD], F32, kind="ExternalInput").ap()
    proj = nc.dram_tensor("proj", [cfg.D, H], F32, kind="ExternalInput").ap()
    W1 = nc.dram_tensor("W1", [H, H], F32, kind="ExternalInput").ap()
    W2 = nc.dram_tensor("W2", [H, H], F32, kind="ExternalInput").ap()
    idx_lo = nc.dram_tensor("idx_lo", [16, NL * 8], mybir.dt.int16,
                            kind="ExternalInput").ap()
    idx_hi = nc.dram_tensor("idx_hi", [16, NH * 8], mybir.dt.int16,
                            kind="ExternalInput").ap()
    S_lo = nc.dram_tensor("S_lo", [128, NL, GRP], F32, kind="ExternalInput").ap()
    S_hi = nc.dram_tensor("S_hi", [128, NH, GRP], F32, kind="ExternalInput").ap()
    out = nc.dram_tensor("out", [NPAD, H], F32, kind="ExternalOutput").ap()
    if not skip_affine:
        gamma = nc.dram_tensor("gamma", [1, H], F32, kind="ExternalInput").ap()
        beta = nc.dram_tensor("beta", [1, H], F32, kind="ExternalInput").ap()

    with tile.TileContext(nc) as tc:
        with (
            tc.tile_pool(name="dram", bufs=1, space="DRAM") as dpool,
            tc.tile_pool(name="singles", bufs=1) as singles,
            tc.tile_pool(name="xt", bufs=2) as xtpool,
            tc.tile_pool(name="pproj", bufs=2, space="PSUM") as pproj,
            tc.tile_pool(name="projsb", bufs=3) as projsb,
            tc.tile_pool(name="glo", bufs=2) as glo,
            tc.tile_pool(name="ghi", bufs=2) as ghi,
            tc.tile_pool(name="slo", bufs=2) as slo,
            tc.tile_pool(name="shi", bufs=2) as shi,
            tc.tile_pool(name="ilo", bufs=2) as ilo,
            tc.tile_pool(name="ihi", bufs=2) as ihi,
            tc.tile_pool(name="paggT", bufs=3, space="PSUM") as paggT,
            tc.tile_pool(name="pout", bufs=3, space="PSUM") as pout,
            tc.tile_pool(name="convsb", bufs=3) as convsb,
            tc.tile_pool(name="ln", bufs=4) as lnpool,
        ):
            h0 = dpool.tile([NPAD, H], F32)
            h1 = dpool.tile([NPAD, H], F32)

            eps_t = singles.tile([128, 1], F32)
            nc.vector.memset(eps_t, cfg.LN_EPS)
            proj_t = singles.tile([128, cfg.D // 128, H], F32)
            nc.sync.dma_start(out=proj_t[:],
                              in_=proj.rearrange("(k p) h -> p k h", p=128))
            W1_t = singles.tile([H, H], F32)
            nc.sync.dma_start(out=W1_t[:], in_=W1[:])
            W2_t = singles.tile([H, H], F32)
            nc.sync.dma_start(out=W2_t[:], in_=W2[:])
            if not skip_affine:
                gamma_t = singles.tile([128, H], F32)
                nc.sync.dma_start(out=gamma_t[:], in_=bass.AP(
                    tensor=gamma.tensor, offset=gamma.offset,
                    ap=[[0, 128]] + gamma.ap[1:]))
                beta_t = singles.tile([128, H], F32)
                nc.sync.dma_start(out=beta_t[:], in_=bass.AP(
                    tensor=beta.tensor, offset=beta.offset,
                    ap=[[0, 128]] + beta.ap[1:]))

            # ---------------- phase A: h0 = x @ proj_c ----------------
            h0_writes = []
            KCH = cfg.D // 128  # contraction chunks
            COLB = 512          # xT column batch
            for c0 in range(0, NPAD, COLB):
                cb = min(COLB, NPAD - c0)
                xts = []
                for k in range(KCH):
                    xt_t = xtpool.tile([128, COLB], F32, tag=f"xt{k}")
                    nc.sync.dma_start(out=xt_t[:, :cb],
                                      in_=xT[k * 128:(k + 1) * 128, c0:c0 + cb])
                    xts.append(xt_t)
                for t0 in range(0, cb, 128):
                    ps = pproj.tile([128, H], F32)
                    for k in range(KCH):
                        nc.tensor.matmul(out=ps[:], lhsT=xts[k][:, t0:t0 + 128],
                                         rhs=proj_t[:, k, :],
                                         start=(k == 0), stop=(k == KCH - 1))
                    sb = projsb.tile([128, H], F32)
                    nc.vector.tensor_copy(out=sb[:], in_=ps[:])
                    h0_writes.append(nc.sync.dma_start(
                        out=h0[c0 + t0:c0 + t0 + 128, :], in_=sb[:]))

            # ---------------- conv layers ----------------
            def conv_layer(lname, h_in, W_t, dst_writer, relu, join_inst):
                tlo = h_in[0:cfg.LO_ROWS, :]
                thi = h_in[cfg.LO_ROWS:NPAD, :]
                s_lo = GatherStream(nc, cfg, f"lo{lname}", idx_lo, S_lo, NL,
                                    tlo, glo, slo, ilo, join_inst)
                s_hi = GatherStream(nc, cfg, f"hi{lname}", idx_hi, S_hi, NH,
                                    thi, ghi, shi, ihi, join_inst)
                ci_lo = ci_hi = 0
                for g in range(cfg.NGRP):
                    nch = (lo_chunks[g], hi_chunks[g])
                    total = nch[0] + nch[1]
                    assert total > 0
                    agg = paggT.tile([H, GRP], F32)
                    done = 0
                    for st, nchunks, ci0 in ((s_lo, nch[0], ci_lo),
                                             (s_hi, nch[1], ci_hi)):
                        for j in range(nchunks):
                            g_ap, s_ap = st.chunk(ci0 + j)
                            nc.tensor.matmul(out=agg[:], lhsT=g_ap, rhs=s_ap,
                                             start=(done == 0),
                                             stop=(done == total - 1))
                            done += 1
                    ci_lo += nch[0]
                    ci_hi += nch[1]

                    aggsb = convsb.tile([H, GRP], F32, tag="aggsb")
                    nc.vector.tensor_copy(out=aggsb[:], in_=agg[:])
                    po = pout.tile([GRP, H], F32)
                    nc.tensor.matmul(out=po[:], lhsT=aggsb[:], rhs=W_t[:],
                                     start=True, stop=True)
                    ob = convsb.tile([GRP, H], F32, tag="ob")
                    nc.vector.tensor_copy(out=ob[:], in_=po[:])
                    # LayerNorm over H (free dim)
                    stats = lnpool.tile([GRP, 6], F32, tag="stats")
                    nc.vector.bn_stats(out=stats[:], in_=ob[:])
                    mv = lnpool.tile([GRP, 2], F32, tag="mv")
                    nc.vector.bn_aggr(out=mv[:], in_=stats[:])
                    rstd = lnpool.tile([GRP, 1], F32, tag="rstd")
                    nc.scalar.activation(out=rstd[:], in_=mv[:, 1:2],
                                         func=mybir.ActivationFunctionType.Sqrt,
                                         bias=eps_t[:GRP, :], scale=1.0)
                    nc.vector.reciprocal(out=rstd[:], in_=rstd[:])
                    nc.vector.tensor_scalar(out=ob[:], in0=ob[:],
                                            scalar1=mv[:, 0:1], scalar2=rstd[:],
                                            op0=mybir.AluOpType.subtract,
                                            op1=mybir.AluOpType.mult)
                    if not skip_affine:
                        nc.vector.tensor_mul(out=ob[:], in0=ob[:],
                                             in1=gamma_t[:GRP, :])
                        nc.vector.tensor_add(out=ob[:], in0=ob[:],
                                             in1=beta_t[:GRP, :])
                    if relu:
                        nc.vector.tensor_scalar_max(out=ob[:], in0=ob[:],
                                                    scalar1=0.0)
                    dst_writer(g, ob)

            h1_writes = []

            def to_h1(g, ob):
                h1_writes.append(
                    nc.sync.dma_start(out=h1[g * GRP:(g + 1) * GRP, :],
                                      in_=ob[:]))

            def to_out(g, ob):
                nc.sync.dma_start(out=out[g * GRP:(g + 1) * GRP, :], in_=ob[:])

            conv_layer("a", h0, W1_t, to_h1, relu=True, join_inst=None)
            conv_layer("b", h1, W2_t, to_out, relu=False, join_inst=None)
            # timing-only: repeat conv work on alternating tables
            def ablation_layer(lname, h_in, with_s, with_mm):
                tlo = h_in[0:cfg.LO_ROWS, :]
                thi = h_in[cfg.LO_ROWS:NPAD, :]
                s_lo = GatherStream(nc, cfg, f"lo{lname}", idx_lo, S_lo, NL,
                                    tlo, glo, slo, ilo, None)
                s_hi = GatherStream(nc, cfg, f"hi{lname}", idx_hi, S_hi, NH,
                                    thi, ghi, shi, ihi, None)
                if not with_s:
                    s_lo.S_dram = s_lo.S_dram[:, 0:1, :]
                    s_hi.S_dram = s_hi.S_dram[:, 0:1, :]

                    def issue_nos(self, bt, _orig=GatherStream._issue):
                        nc2, cfg2 = self.nc, self.cfg
                        b0 = bt * cfg2.BH
                        B = min(cfg2.BH, self.n_chunks - b0)
                        itile = self.ipool.tile([128, cfg2.BH * 8],
                                                mybir.dt.int16, tag="i")
                        idx_src = self.idx_dram[:, b0 * 8: b0 * 8 + B * 8]
                        bc = bass.AP(tensor=idx_src.tensor,
                                     offset=idx_src.offset,
                                     ap=[[0, 8]] + idx_src.ap)
                        nc2.sync.dma_start(out=itile[:, :B * 8], in_=bc)
                        self.gtile = self.gpool.tile(
                            [128, cfg2.BH, cfg2.H], F32, tag="g")
                        for cb in range(0, B, cfg2.CALLB):
                            nb = min(cfg2.CALLB, B - cb)
                            nc2.gpsimd.dma_gather(
                                out_ap=self.gtile[:, cb:cb + nb, :],
                                in_ap=self.table_view,
                                idxs_ap=itile[:, cb * 8:(cb + nb) * 8],
                                num_idxs=nb * 128, num_idxs_reg=nb * 128,
                                elem_size=cfg2.H,
                                single_packet=cfg2.single_packet)
                        self.stile = self.spool.tile(
                            [128, cfg2.BH, cfg2.GRP], F32, tag="s")
                        self.cur_bt = bt
                    s_lo._issue = issue_nos.__get__(s_lo)
                    s_hi._issue = issue_nos.__get__(s_hi)
                ci_lo = ci_hi = 0
                for g in range(cfg.NGRP):
                    nch = (lo_chunks[g], hi_chunks[g])
                    total = nch[0] + nch[1]
                    if with_mm:
                        agg = paggT.tile([H, GRP], F32)
                    done = 0
                    for st, nchunks, ci0 in ((s_lo, nch[0], ci_lo),
                                             (s_hi, nch[1], ci_hi)):
                        for j in range(nchunks):
                            g_ap, s_ap = st.chunk(ci0 + j)
                            if with_mm:
                                nc.tensor.matmul(out=agg[:], lhsT=g_ap,
                                                 rhs=s_ap,
                                                 start=(done == 0),
                                                 stop=(done == total - 1))
                            elif j == 0:
                                tch = lnpool.tile([128, 1], F32, tag="touch")
                                nc.vector.tensor_copy(out=tch[:],
                                                      in_=g_ap[:, 0:1])
                            done += 1
                    ci_lo += nch[0]
                    ci_hi += nch[1]
                    if with_mm:
                        aggsb = convsb.tile([H, GRP], F32, tag="aggsb")
                        nc.vector.tensor_copy(out=aggsb[:], in_=agg[:])

            tabs = [h1, h0]
            for i in range(extra_layers):
                hsrc, hdst = tabs[i % 2], tabs[(i + 1) % 2]
                if flavor == "full":
                    def wr(g, ob, hdst=hdst):
                        nc.sync.dma_start(
                            out=hdst[g * GRP:(g + 1) * GRP, :], in_=ob[:])
                    conv_layer(f"x{i}", hsrc, W1_t, wr, relu=True,
                               join_inst=None)
                else:
                    ablation_layer(f"x{i}", hsrc, with_s=(flavor != "g"),
                                   with_mm=(flavor == "gsm"))

    nc.compile()
    return nc


# ----------------------------------------------------------------------------
# Entry point
# ----------------------------------------------------------------------------

def kernel(x, edge_index, omega, proj, W1, W2, ln_gamma, ln_beta):
    cfg = FULL
    per_core, lo_chunks, hi_chunks, skip_affine = preprocess(
        cfg, x, edge_index, omega, proj, W1, W2, ln_gamma, ln_beta)
    nc = build_program(cfg, lo_chunks, hi_chunks, skip_affine,
                       num_devices=cfg.n_cores)
    res = bass_utils.run_bass_kernel_spmd(
        nc, per_core, core_ids=list(range(cfg.n_cores)))
    out = np.stack([res.results[c]["out"][:cfg.N] for c in range(cfg.C)],
                   axis=1)
    return np.ascontiguousarray(out, dtype=np.float32)



# revision 3
# speedup vs baseline: 2.9883x; 2.9883x over previous
"""Trainium2 Bass kernel for nn_DisentangledGraphConvEncoder (v2).

Strategy: dst-node sharding in bf16. Core c owns nodes [c*6272, (c+1)*6272).
Each conv layer gathers full 1KB rows (all C=8 channels of a node) for its
~100k edges, multiplies by per-(edge,channel) omega on DVE (broadcast over H),
scatter-adds via PE matmuls against a shared one-hot S (128-wide dst groups),
applies the per-channel weight matmul + LayerNorm, and writes its node slice.
An 8-core AllGather rebuilds the full feature table between layers.

vs the channel-sharded baseline this cuts gather descriptors 9x (1KB rows vs
256B), halves gathered bytes (bf16), and cuts S-matrix traffic 16x (shared
across channels, bf16), at the cost of two small (6.4MB/rank) AllGathers.
"""

from dataclasses import dataclass

import numpy as np
import ml_dtypes

import concourse.bass as bass
import concourse.bacc as bacc
import concourse.tile as tile
from concourse import mybir
from concourse import bass_utils

BF16 = mybir.dt.bfloat16
FP8 = mybir.dt.float8e4
F32 = mybir.dt.float32
I16 = mybir.dt.int16
NPBF16 = ml_dtypes.bfloat16
NPFP8 = ml_dtypes.float8_e4m3


@dataclass
class Cfg:
    N: int = 50000
    E: int = 800000
    D: int = 256
    C: int = 8
    H: int = 64
    GRP: int = 128         # one-hot width / dst group size
    LO_ROWS: int = 25600   # lo gather-table rows (int16 idx limit)
    BH: int = 16           # chunks per gather batch (= one dma_gather call)
    LN_EPS: float = 1e-5
    n_cores: int = 8
    dma_scratch: int = 65536

    @property
    def CH(self):
        return self.C * self.H          # 512

    @property
    def PER(self):
        return 6272                      # nodes per core (= 49 * 128)

    @property
    def NPAD(self):
        return self.PER * self.n_cores   # 50176

    @property
    def NGRP(self):
        return self.PER // self.GRP      # 49 local groups


FULL = Cfg()


# ----------------------------------------------------------------------------
# Host-side preprocessing
# ----------------------------------------------------------------------------

def _build_core_half(cfg, sched, src_vals, g_of, col_of, om_vals, lo):
    """Build padded idx16 / S / om arrays for one (core, half) edge stream.

    sched: common chunks-per-group schedule [NGRP]. Edges are dst-sorted.
    """
    nchunks = int(sched.sum())
    L = nchunks * 128
    start_slot = np.concatenate([[0], np.cumsum(sched)[:-1]]) * 128
    counts = np.bincount(g_of, minlength=cfg.NGRP)
    cum = np.concatenate([[0], np.cumsum(counts)[:-1]])
    rank = np.arange(len(g_of)) - np.repeat(cum, counts)
    pos = start_slot[g_of] + rank

    idx_vals = np.zeros(L, np.int64)
    idx_vals[pos] = src_vals if lo else src_vals - cfg.LO_ROWS
    assert idx_vals.min() >= 0 and idx_vals.max() < 32768
    idx16 = np.ascontiguousarray(idx_vals.astype(np.int16).reshape(-1, 16).T)

    S = np.zeros((L, cfg.GRP), NPFP8)
    S[pos, col_of] = 1.0
    S = np.ascontiguousarray(S.reshape(nchunks, 128, cfg.GRP).transpose(1, 0, 2))

    om = np.zeros((L, cfg.C), np.float32)
    om[pos] = om_vals
    om = np.ascontiguousarray(
        om.reshape(nchunks, 128, cfg.C).transpose(1, 0, 2)).astype(NPBF16)
    return idx16, S, om


def preprocess(cfg, x, edge_index, omega, proj, W1, W2, ln_gamma, ln_beta):
    src = np.asarray(edge_index[0], dtype=np.int64)
    dst = np.asarray(edge_index[1], dtype=np.int64)
    omega = np.asarray(omega, dtype=np.float32)
    x = np.asarray(x, dtype=np.float32)

    order = np.argsort(dst, kind="stable")
    src_s, dst_s, om_s = src[order], dst[order], omega[order]

    core_of = dst_s // cfg.PER
    loc = dst_s % cfg.PER
    g_of = loc // cfg.GRP
    col_of = loc % cfg.GRP
    lo_mask = src_s < cfg.LO_ROWS

    # common per-group chunk schedule (max over cores)
    cnt = np.zeros((cfg.n_cores, 2, cfg.NGRP), np.int64)
    np.add.at(cnt, (core_of, (~lo_mask).astype(np.int64), g_of), 1)
    chunks = -(-cnt // 128)                       # ceil
    sched_lo = chunks[:, 0, :].max(axis=0)
    sched_hi = chunks[:, 1, :].max(axis=0)
    sched_lo = np.maximum(sched_lo, (sched_lo + sched_hi == 0).astype(np.int64))

    xT = np.zeros((cfg.D, cfg.NPAD), np.float32)
    xT[:, :cfg.N] = x.T
    xT = xT.astype(NPBF16)
    # feature-row layout (p, h, cl): row col p*128 + h*2 + cl holds channel
    # c = 2p + cl. Gives stride-1 innermost APs for the omega broadcast and
    # single-free-dim matmul lhsT slices.
    perm = np.empty(cfg.CH, np.int64)
    for c in range(cfg.C):
        p, cl = c // 2, c % 2
        perm[p * 128 + np.arange(cfg.H) * 2 + cl] = c * cfg.H + np.arange(cfg.H)
    projh = np.asarray(proj, np.float32).reshape(cfg.D, cfg.CH)[:, perm]
    projh = np.ascontiguousarray(projh).astype(NPBF16)

    def wprep(W):
        # parity-zeroed weights: Wz[2h+cl, p, cl*64+hout] = W[2p+cl][h, hout]
        W = np.asarray(W, np.float32)
        Wz = np.zeros((128, 4, 128), np.float32)
        for p in range(4):
            for cl in range(2):
                Wz[cl::2, p, cl * 64:(cl + 1) * 64] = W[2 * p + cl]
        return np.ascontiguousarray(Wz).astype(NPBF16)

    skip_affine = bool(np.all(np.asarray(ln_gamma) == 1.0)
                       and np.all(np.asarray(ln_beta) == 0.0))

    per_core = []
    for c in range(cfg.n_cores):
        m = {
            "xTs": np.ascontiguousarray(
                xT[:, c * cfg.PER:(c + 1) * cfg.PER]),
            "proj": projh,
            "W1": wprep(W1),
            "W2": wprep(W2),
        }
        for half, lo in (("lo", True), ("hi", False)):
            sel = np.nonzero((core_of == c) & (lo_mask == lo))[0]
            idx16, S, om = _build_core_half(
                cfg, sched_lo if lo else sched_hi,
                src_s[sel], g_of[sel], col_of[sel], om_s[sel], lo)
            m[f"idx_{half}"] = idx16
            m[f"S_{half}"] = S
            m[f"om_{half}"] = om
        if not skip_affine:
            m["gamma"] = np.asarray(ln_gamma, np.float32).reshape(1, cfg.H)
            m["beta"] = np.asarray(ln_beta, np.float32).reshape(1, cfg.H)
        per_core.append(m)

    return (per_core, [int(v) for v in sched_lo], [int(v) for v in sched_hi],
            skip_affine)


# ----------------------------------------------------------------------------
# Bass program
# ----------------------------------------------------------------------------

class GatherStream:
    """Streams gathered-row tiles (omega-premultiplied) + S tiles for one
    (layer, half)."""

    def __init__(self, nc, cfg, idx_dram, S_dram, om_dram, n_chunks,
                 table_view, gpool, spool, opool, ipool, tag):
        self.nc, self.cfg = nc, cfg
        self.idx_dram, self.S_dram, self.om_dram = idx_dram, S_dram, om_dram
        self.n_chunks = n_chunks
        self.table_view = table_view
        self.gpool, self.spool, self.opool, self.ipool = \
            gpool, spool, opool, ipool
        self.tag = tag
        self.cur_bt = -1
        self.gtile = None
        self.stile = None

    def _issue(self, bt):
        nc, cfg = self.nc, self.cfg
        b0 = bt * cfg.BH
        B = min(cfg.BH, self.n_chunks - b0)

        itile = self.ipool.tile([128, cfg.BH * 8], I16, tag=f"i{self.tag}")
        idx_src = self.idx_dram[:, b0 * 8: b0 * 8 + B * 8]
        bcast = bass.AP(tensor=idx_src.tensor, offset=idx_src.offset,
                        ap=[[0, 8]] + idx_src.ap)
        nc.sync.dma_start(out=itile[:, :B * 8], in_=bcast)

        self.gtile = self.gpool.tile([128, cfg.BH, cfg.CH], BF16,
                                     tag=f"g{self.tag}")
        nc.gpsimd.dma_gather(
            out_ap=self.gtile[:, :B, :], in_ap=self.table_view,
            idxs_ap=itile[:, :B * 8],
            num_idxs=B * 128, num_idxs_reg=B * 128,
            elem_size=cfg.CH, single_packet=False)

        self.stile = self.spool.tile([128, cfg.BH, cfg.GRP], FP8,
                                     tag=f"s{self.tag}")
        nc.sync.dma_start(out=self.stile[:, :B, :],
                          in_=self.S_dram[:, b0:b0 + B, :])
        otile = self.opool.tile([128, cfg.BH, cfg.C], BF16,
                                tag=f"o{self.tag}")
        nc.sync.dma_start(out=otile[:, :B, :],
                          in_=self.om_dram[:, b0:b0 + B, :])

        # msg = g * omega (broadcast over H), in place on the gather tile.
        # (p,h,cl) row layout: om iterated as (b*p merged, h, cl) with
        # stride-1 innermost on every operand -> DVE 2x 16-bit mode.
        gv = self.gtile[:, :B, :]
        ov = otile[:, :B, :]
        om4 = bass.AP(tensor=otile.tensor, offset=ov.offset,
                      ap=[ov.ap[0], [2, 4 * B], [0, cfg.H], [1, 2]])
        nc.vector.tensor_mul(out=gv, in0=gv, in1=om4)
        self.cur_bt = bt

    def chunk(self, ci):
        bt, off = divmod(ci, self.cfg.BH)
        if bt != self.cur_bt:
            assert bt == self.cur_bt + 1
            self._issue(bt)
        return self.gtile[:, off, :], self.stile[:, off, :]


def build_program(cfg, sched_lo, sched_hi, skip_affine, num_devices=8,
                  no_collective=False):
    nc = bacc.Bacc("TRN2", target_bir_lowering=False, debug=False,
                   num_devices=num_devices,
                   dynamic_dma_scratch_size=cfg.dma_scratch)
    C, H, CH, GRP, PER, NPAD = cfg.C, cfg.H, cfg.CH, cfg.GRP, cfg.PER, cfg.NPAD
    NL, NH = sum(sched_lo), sum(sched_hi)

    xTs = nc.dram_tensor("xTs", [cfg.D, PER], BF16, kind="ExternalInput").ap()
    proj = nc.dram_tensor("proj", [cfg.D, CH], BF16, kind="ExternalInput").ap()
    W1 = nc.dram_tensor("W1", [128, 4, 128], BF16, kind="ExternalInput").ap()
    W2 = nc.dram_tensor("W2", [128, 4, 128], BF16, kind="ExternalInput").ap()
    idx_lo = nc.dram_tensor("idx_lo", [16, NL * 8], I16, kind="ExternalInput").ap()
    idx_hi = nc.dram_tensor("idx_hi", [16, NH * 8], I16, kind="ExternalInput").ap()
    S_lo = nc.dram_tensor("S_lo", [128, NL, GRP], FP8, kind="ExternalInput").ap()
    S_hi = nc.dram_tensor("S_hi", [128, NH, GRP], FP8, kind="ExternalInput").ap()
    om_lo = nc.dram_tensor("om_lo", [128, NL, C], BF16, kind="ExternalInput").ap()
    om_hi = nc.dram_tensor("om_hi", [128, NH, C], BF16, kind="ExternalInput").ap()
    out = nc.dram_tensor("out", [PER, C, H], F32, kind="ExternalOutput").ap()
    if not skip_affine:
        gamma = nc.dram_tensor("gamma", [1, H], F32, kind="ExternalInput").ap()
        beta = nc.dram_tensor("beta", [1, H], F32, kind="ExternalInput").ap()

    from contextlib import ExitStack
    with tile.TileContext(nc) as tc, ExitStack() as _stk:
        def pool(*a, **k):
            return _stk.enter_context(tc.tile_pool(*a, **k))
        dpool = pool(name="dram", bufs=1, space="DRAM")
        singles = pool(name="singles", bufs=1)
        xtpool = pool(name="xt", bufs=2)
        pproj = pool(name="pproj", bufs=2, space="PSUM")
        projsb = pool(name="projsb", bufs=3)
        glo = pool(name="glo", bufs=3)
        ghi = pool(name="ghi", bufs=3)
        slo = pool(name="slo", bufs=3)
        shi = pool(name="shi", bufs=3)
        olo = pool(name="olo", bufs=3)
        ohi = pool(name="ohi", bufs=3)
        ilo = pool(name="ilo", bufs=3)
        ihi = pool(name="ihi", bufs=3)
        pagg = pool(name="pagg", bufs=3, space="PSUM")
        ppo = pool(name="ppo", bufs=3, space="PSUM")
        aggsbp = pool(name="aggsb", bufs=3)
        obpool = pool(name="ob", bufs=3)
        sqpool = pool(name="sq", bufs=2)
        lnpool = pool(name="ln", bufs=6)
        if True:
            h0s = dpool.tile([PER, CH], BF16)
            h1s = dpool.tile([PER, CH], BF16)
            h0f = dpool.tile([NPAD, CH], BF16, addr_space="Shared")
            h1f = dpool.tile([NPAD, CH], BF16, addr_space="Shared")

            eps_t = singles.tile([128, 1], F32)
            nc.vector.memset(eps_t, cfg.LN_EPS)
            proj_t = singles.tile([128, cfg.D // 128, CH], BF16)
            nc.sync.dma_start(out=proj_t[:],
                              in_=proj.rearrange("(k p) h -> p k h", p=128))
            W1_t = singles.tile([128, 4, 128], BF16)
            nc.sync.dma_start(out=W1_t[:], in_=W1[:])
            W2_t = singles.tile([128, 4, 128], BF16)
            nc.sync.dma_start(out=W2_t[:], in_=W2[:])
            if not skip_affine:
                gamma_t = singles.tile([128, H], F32)
                nc.sync.dma_start(out=gamma_t[:], in_=bass.AP(
                    tensor=gamma.tensor, offset=gamma.offset,
                    ap=[[0, 128]] + gamma.ap[1:]))
                beta_t = singles.tile([128, H], F32)
                nc.sync.dma_start(out=beta_t[:], in_=bass.AP(
                    tensor=beta.tensor, offset=beta.offset,
                    ap=[[0, 128]] + beta.ap[1:]))

            # ---------------- phase A: h0 slice = x_slice @ proj ----------
            KCH = cfg.D // 128
            COLB = 512
            xTr = xTs.rearrange("(k p) n -> p k n", p=128)
            for c0 in range(0, PER, COLB):
                cb = min(COLB, PER - c0)
                xt_t = xtpool.tile([128, KCH, COLB], BF16, tag="xt")
                nc.sync.dma_start(out=xt_t[:, :, :cb],
                                  in_=xTr[:, :, c0:c0 + cb])
                for t0 in range(0, cb, 128):
                    ps = pproj.tile([128, CH], F32)
                    for k in range(KCH):
                        nc.tensor.matmul(out=ps[:],
                                         lhsT=xt_t[:, k, t0:t0 + 128],
                                         rhs=proj_t[:, k, :],
                                         start=(k == 0), stop=(k == KCH - 1))
                    sb = projsb.tile([128, CH], BF16, tag="psb")
                    nc.scalar.activation(
                        out=sb[:], in_=ps[:],
                        func=mybir.ActivationFunctionType.Copy)
                    nc.sync.dma_start(
                        out=h0s[c0 + t0:c0 + t0 + 128, :], in_=sb[:])

            def allgather(src_t, dst_t):
                if no_collective:
                    nc.sync.dma_start(out=dst_t[0:PER, :], in_=src_t[:])
                else:
                    nc.gpsimd.collective_compute(
                        "AllGather", mybir.AluOpType.bypass,
                        replica_groups=[list(range(num_devices))],
                        ins=[src_t.opt()], outs=[dst_t.opt()])

            allgather(h0s, h0f)

            # ---------------- conv layers ----------------
            def conv_layer(tag, h_in, W_t, writer, relu):
                tlo = h_in[0:cfg.LO_ROWS, :]
                thi = h_in[cfg.LO_ROWS:NPAD, :]
                s_lo = GatherStream(nc, cfg, idx_lo, S_lo, om_lo, NL, tlo,
                                    glo, slo, olo, ilo, "lo")
                s_hi = GatherStream(nc, cfg, idx_hi, S_hi, om_hi, NH, thi,
                                    ghi, shi, ohi, ihi, "hi")
                ci_lo = ci_hi = 0
                relu_f = (mybir.ActivationFunctionType.Relu if relu
                          else mybir.ActivationFunctionType.Copy)
                for g in range(cfg.NGRP):
                    nlo, nhi = sched_lo[g], sched_hi[g]
                    total = nlo + nhi
                    agg = pagg.tile([128, 4, GRP], F32)
                    done = 0
                    for st, nch, ci0 in ((s_lo, nlo, ci_lo),
                                         (s_hi, nhi, ci_hi)):
                        for j in range(nch):
                            g_ap, s_ap = st.chunk(ci0 + j)
                            for p in range(4):
                                nc.tensor.matmul(
                                    out=agg[:, p, :],
                                    lhsT=g_ap[:, p * 128:(p + 1) * 128],
                                    rhs=s_ap,
                                    start=(done == 0 and p == 0),
                                    stop=(done == total - 1 and p == 3))
                            done += 1
                    ci_lo += nlo
                    ci_hi += nhi

                    aggsb = aggsbp.tile([128, 4, GRP], BF16, tag="aggsb")
                    nc.scalar.activation(
                        out=aggsb[:], in_=agg[:],
                        func=mybir.ActivationFunctionType.Copy)

                    po = ppo.tile([128, C, H], F32)
                    for p in range(4):
                        nc.tensor.matmul(
                            out=po[:, 2 * p:2 * p + 2, :],
                            lhsT=aggsb[:, p, :],
                            rhs=W_t[:, p, :],
                            start=(p == 0), stop=(p == 3))

                    ob = obpool.tile([128, C, H], F32 if not relu else BF16,
                                     tag=f"ob{tag}")
                    # batched LN stats over all 8 channels:
                    # mean/var via sum and sum-of-squares reduces over H
                    sum8 = lnpool.tile([128, C], F32, tag="sum8")
                    nc.vector.tensor_reduce(out=sum8[:], in_=po[:],
                                            axis=mybir.AxisListType.X,
                                            op=mybir.AluOpType.add)
                    sq = sqpool.tile([128, C, H], F32, tag="sq")
                    nc.scalar.activation(
                        out=sq[:], in_=po[:],
                        func=mybir.ActivationFunctionType.Square)
                    ss8 = lnpool.tile([128, C], F32, tag="ss8")
                    nc.vector.tensor_reduce(out=ss8[:], in_=sq[:],
                                            axis=mybir.AxisListType.X,
                                            op=mybir.AluOpType.add)
                    m8 = lnpool.tile([128, C], F32, tag="m8")
                    nc.vector.tensor_scalar(out=m8[:], in0=sum8[:],
                                            scalar1=1.0 / H, scalar2=None,
                                            op0=mybir.AluOpType.mult)
                    musq = lnpool.tile([128, C], F32, tag="musq")
                    nc.vector.tensor_mul(out=musq[:], in0=m8[:], in1=m8[:])
                    var8 = lnpool.tile([128, C], F32, tag="var8")
                    nc.vector.scalar_tensor_tensor(
                        out=var8[:], in0=ss8[:], scalar=1.0 / H,
                        in1=musq[:], op0=mybir.AluOpType.mult,
                        op1=mybir.AluOpType.subtract)
                    rstd8 = lnpool.tile([128, C], F32, tag="rstd8")
                    nc.scalar.activation(out=rstd8[:], in_=var8[:],
                                         func=mybir.ActivationFunctionType.Sqrt,
                                         bias=eps_t[:], scale=1.0)
                    nc.vector.reciprocal(out=rstd8[:], in_=rstd8[:])
                    if skip_affine and relu:
                        nb8 = lnpool.tile([128, C], F32, tag="nb8")
                        nc.vector.scalar_tensor_tensor(
                            out=nb8[:], in0=m8[:], scalar=-1.0, in1=rstd8[:],
                            op0=mybir.AluOpType.mult, op1=mybir.AluOpType.mult)
                        obv = ob[:]
                        for c in range(C):
                            p, cl = c // 2, c % 2
                            oc = bass.AP(tensor=ob.tensor,
                                         offset=obv.offset + p * 128 + cl,
                                         ap=[obv.ap[0], [2, H]])
                            nc.scalar.activation(
                                out=oc, in_=po[:, c, :],
                                func=relu_f, bias=nb8[:, c:c + 1],
                                scale=rstd8[:, c:c + 1])
                    elif skip_affine:
                        # (po - mu) * rstd with per-(partition,channel)
                        # scalars, done as two H-broadcast tensor ops
                        def bc(t):
                            v = t[:]
                            return bass.AP(tensor=t.tensor, offset=v.offset,
                                           ap=[v.ap[0], v.ap[1], [0, H]])
                        cen = sqpool.tile([128, C, H], F32, tag="cen")
                        nc.vector.tensor_sub(out=cen[:], in0=po[:],
                                             in1=bc(m8))
                        nc.vector.tensor_mul(out=ob[:], in0=cen[:],
                                             in1=bc(rstd8))
                    else:
                        for c in range(C):
                            nc.vector.tensor_scalar(
                                out=ob[:, c, :], in0=po[:, c, :],
                                scalar1=m8[:, c:c + 1],
                                scalar2=rstd8[:, c:c + 1],
                                op0=mybir.AluOpType.subtract,
                                op1=mybir.AluOpType.mult)
                            if True:
                                nc.vector.tensor_mul(out=ob[:, c, :],
                                                     in0=ob[:, c, :],
                                                     in1=gamma_t[:, :])
                                nc.vector.tensor_add(out=ob[:, c, :],
                                                     in0=ob[:, c, :],
                                                     in1=beta_t[:, :])
                                if relu:
                                    nc.vector.tensor_scalar_max(
                                        out=ob[:, c, :], in0=ob[:, c, :],
                                        scalar1=0.0)
                    writer(g, ob)

            def to_h1(g, ob):
                flat = bass.AP(tensor=ob.tensor, offset=ob[:].offset,
                               ap=[ob[:].ap[0], [1, CH]])
                nc.sync.dma_start(out=h1s[g * GRP:(g + 1) * GRP, :], in_=flat)

            def to_out(g, ob):
                nc.sync.dma_start(out=out[g * GRP:(g + 1) * GRP, :, :],
                                  in_=ob[:])

            conv_layer("a", h0f, W1_t, to_h1, relu=True)
            allgather(h1s, h1f)
            conv_layer("b", h1f, W2_t, to_out, relu=False)

    nc.compile()
    return nc


# ----------------------------------------------------------------------------
# Entry point
# ----------------------------------------------------------------------------

def build_for_sim(inputs, no_collective=True):
    cfg = FULL
    per_core, sched_lo, sched_hi, skip_affine = preprocess(cfg, **inputs)
    nc = build_program(cfg, sched_lo, sched_hi, skip_affine,
                       num_devices=cfg.n_cores, no_collective=no_collective)
    return nc, per_core


def kernel(x, edge_index, omega, proj, W1, W2, ln_gamma, ln_beta):
    cfg = FULL
    per_core, sched_lo, sched_hi, skip_affine = preprocess(
        cfg, x, edge_index, omega, proj, W1, W2, ln_gamma, ln_beta)
    nc = build_program(cfg, sched_lo, sched_hi, skip_affine,
                       num_devices=cfg.n_cores)
    res = bass_utils.run_bass_kernel_spmd(
        nc, per_core, core_ids=list(range(cfg.n_cores)))
    out = np.concatenate([res.results[c]["out"] for c in range(cfg.n_cores)],
                         axis=0)[:cfg.N]
    return np.ascontiguousarray(out, dtype=np.float32)
